# revision 4
# baseline (speedup 1.0000x reference)
"""Causal self-attention (B=4, T=2048, C=768, H=6, D=128) on 8 trn2 NeuronCores.

Sharding: 24 (batch, head) units -> 8 cores, each core owns 1 batch x 3 heads.
Per core: QKV projections for its 3 heads, RoPE + per-head norm, causal
attention, partial output projection over its heads' columns.
Unshard: out[b] = partial[core 2b] + partial[core 2b+1]  (tensor-parallel sum).

v2 (bf16 rebuild of the fp32r baseline; trace showed the PE running fp32
matmuls at 2 cyc/col with ~200ns LDWEIGHTS each and the ACT engine 85% busy):
  - every matmul operand is bf16: 1 cyc/col streaming + fast-weight-load,
    halving PE time; PSUM accumulation stays fp32 so only input quantization
    (~0.2-0.6% here, gate is 2e-2) is lost.
  - rope via host tables CC=[cos|cos], SS=[sin|-sin]: 4 DVE tensor ops per
    (tile, q/k) instead of 6, all bf16 at 2x mode.
  - q/k stats via bn_stats (one DVE op per q/k) + tiny [128,6] combines;
    rstd = exp(-0.5*ln(var/127)) so the only ACT tables used all kernel are
    {Exp, Ln, Copy} = one table set, no mid-kernel table loads. eps=1e-6 is
    dropped (std ~ 0.55, relative effect 2e-6).
  - norm-apply = one tensor_scalar (r*rstd + (-mean*rstd)) per head, bf16 4x.
  - exp over PAIRED score blocks [128, 1024] (fewer 352-cycle ACT overheads).
  - causal mask via a host [128, 896] 0/1 bf16 sliding-window table: one DVE
    multiply per diagonal block (replaces gpsimd affine_select).
  - softmax denominator reciprocal via reciprocal_approx_fast (5x cheaper
    than the iterative DVE reciprocal).
  - partial outputs leave the device in bf16; host sums core pairs in fp32.
"""

import ml_dtypes
import numpy as np

import concourse.bacc as bacc
import concourse.bass as bass
import concourse.mybir as mybir
from concourse import tile
from concourse.bass_utils import run_bass_kernel_spmd

F32 = mybir.dt.float32
BF16 = mybir.dt.bfloat16
AF = mybir.ActivationFunctionType
ALU = mybir.AluOpType

B, T, C, H, D = 4, 2048, 768, 6, 128
HALF = D // 2
NH = 3            # heads per core
CT = C // 128     # 6 contraction tiles for projections
NT = T // 128     # 16 token tiles
QC = 512          # query-chunk width for attention
NQC = T // QC     # 4 chunks
SCALE = 1.0 / float(np.sqrt(D))

_CACHE = {}


def _build_nc():
    nc = bacc.Bacc("TRN2")

    xT = nc.dram_tensor("xT", [C, T], BF16, kind="ExternalInput")
    wqT = nc.dram_tensor("wqT", [C, NH * D], BF16, kind="ExternalInput")
    wkT = nc.dram_tensor("wkT", [C, NH * D], BF16, kind="ExternalInput")
    wvT = nc.dram_tensor("wvT", [C, NH * D], BF16, kind="ExternalInput")
    wpT = nc.dram_tensor("wpT", [NH * D, C], BF16, kind="ExternalInput")
    ccin = nc.dram_tensor("ccin", [T, NH * D], BF16, kind="ExternalInput")
    ssin = nc.dram_tensor("ssin", [T, NH * D], BF16, kind="ExternalInput")
    ident = nc.dram_tensor("ident", [128, 128], BF16, kind="ExternalInput")
    ones_in = nc.dram_tensor("ones_in", [128, 1], BF16, kind="ExternalInput")
    mask_in = nc.dram_tensor("mask_in", [128, 384 + QC], BF16, kind="ExternalInput")
    out = nc.dram_tensor("out", [T, C], BF16, kind="ExternalOutput")

    with tile.TileContext(nc) as tc:
        with (
            tc.tile_pool(name="persist", bufs=1) as persist,
            tc.tile_pool(name="qkvbuf", bufs=1) as qkvbuf,
        ):
            QT = qkvbuf.tile([128, NH, T], BF16)       # [d, h, t]
            KT = qkvbuf.tile([128, NH, T], BF16)       # [d, h, t]
            V = qkvbuf.tile([128, NT, NH * D], BF16)   # [s%128, s//128, h*D+d]
            ones = persist.tile([128, 1], BF16)
            idn = persist.tile([128, 128], BF16)
            wp_sb = persist.tile([128, NH, C], BF16)   # [d, h, c]
            msk = persist.tile([128, 384 + QC], BF16)

            # ---------------- stage 1+2: QKV projection + rope + norm ---------
            with (
                tc.tile_pool(name="wbuf", bufs=1) as wbuf,
                tc.tile_pool(name="xch", bufs=3) as xpool,
                tc.tile_pool(name="rope", bufs=4) as rpool,
                tc.tile_pool(name="stat", bufs=4) as spool,
                tc.tile_pool(name="psA", bufs=3, space="PSUM") as psA,
                tc.tile_pool(name="psT", bufs=2, space="PSUM") as psT,
            ):
                wq_sb = wbuf.tile([128, CT, NH * D], BF16)
                wk_sb = wbuf.tile([128, CT, NH * D], BF16)
                wv_sb = wbuf.tile([128, CT, NH * D], BF16)
                # startup-latency ordering: first-tile deps (weights, x tile 0)
                # are issued first; CC/SS next; attention-only tensors last
                nc.sync.dma_start(wq_sb[:], wqT.rearrange("(ci p) o -> p ci o", p=128))
                nc.sync.dma_start(wk_sb[:], wkT.rearrange("(ci p) o -> p ci o", p=128))
                nc.sync.dma_start(wv_sb[:], wvT.rearrange("(ci p) o -> p ci o", p=128))

                xT_r = xT.rearrange("(ci p) (tt t) -> p ci tt t", p=128, t=128)
                xch0 = xpool.tile([128, CT, 128], BF16, tag="xch")
                nc.sync.dma_start(xch0[:], xT_r[:, :, 0, :])

                cc_sb = wbuf.tile([128, NT, NH * D], BF16)
                ss_sb = wbuf.tile([128, NT, NH * D], BF16)
                nc.sync.dma_start(cc_sb[:], ccin.rearrange("(tt p) f -> p tt f", p=128))
                nc.sync.dma_start(ss_sb[:], ssin.rearrange("(tt p) f -> p tt f", p=128))
                nc.sync.dma_start(idn[:], ident[:])
                nc.sync.dma_start(wp_sb[:], wpT.rearrange("(h p) c -> p h c", p=128))
                nc.sync.dma_start(ones[:], ones_in[:])
                nc.sync.dma_start(msk[:], mask_in[:])

                for tt in range(NT):
                    if tt == 0:
                        xch = xch0
                    else:
                        xch = xpool.tile([128, CT, 128], BF16, tag="xch")
                        nc.sync.dma_start(xch[:], xT_r[:, :, tt, :])

                    qps = psA.tile([128, NH * D], F32, tag="ps")
                    kps = psA.tile([128, NH * D], F32, tag="ps")
                    vps = psA.tile([128, NH * D], F32, tag="ps")
                    for ci in range(CT):
                        st_, sp_ = (ci == 0), (ci == CT - 1)
                        lhs = xch[:, ci, :]
                        nc.tensor.matmul(qps[:], lhs, wq_sb[:, ci, :], start=st_, stop=sp_)
                        nc.tensor.matmul(kps[:], lhs, wk_sb[:, ci, :], start=st_, stop=sp_)
                        nc.tensor.matmul(vps[:], lhs, wv_sb[:, ci, :], start=st_, stop=sp_)

                    # V: straight copy PSUM -> SBUF bf16 in natural [t, o] layout
                    nc.scalar.copy(V[:, tt, :], vps[:])

                    cc_t = cc_sb[:, tt].rearrange("p (h d) -> p h d", h=NH)
                    ss_t = ss_sb[:, tt].rearrange("p (h d) -> p h d", h=NH)

                    # stats tile: [q/k, head, (cnt, mean, cnt*var) x even/odd]
                    S = spool.tile([128, 2, NH, 6], F32, tag="S")
                    rr = []
                    for mi, ps in enumerate((qps, kps)):
                        sb = rpool.tile([128, NH * D], BF16, tag=f"sb{mi}")
                        nc.scalar.copy(sb[:], ps[:])
                        sb_v = sb[:].rearrange("p (h d) -> p h d", h=NH)
                        r = rpool.tile([128, NH * D], BF16, tag=f"r{mi}")
                        rr.append(r)
                        r_v = r[:].rearrange("p (h d) -> p h d", h=NH)
                        t2 = rpool.tile([128, NH * D], BF16, tag=f"t2{mi}")
                        t2_v = t2[:].rearrange("p (h d) -> p h d", h=NH)
                        # rope: r = u*CC + swap(u)*SS, swap done by half-slices
                        nc.vector.tensor_mul(
                            t2_v[:, :, 0:HALF], sb_v[:, :, HALF:D], ss_t[:, :, 0:HALF])
                        nc.vector.tensor_mul(
                            t2_v[:, :, HALF:D], sb_v[:, :, 0:HALF], ss_t[:, :, HALF:D])
                        nc.vector.tensor_mul(r[:], sb[:], cc_sb[:, tt, :])
                        nc.vector.tensor_add(r[:], r[:], t2[:])
                        for h in range(NH):
                            nc.vector.bn_stats(S[:, mi, h], r_v[:, h])

                    # var*128 = cv_e + cv_o + 32*(m_e - m_o)^2   (ddof=1 -> /127)
                    dm = spool.tile([128, 2 * NH], F32, tag="dm")
                    ms = spool.tile([128, 2 * NH], F32, tag="ms")
                    cv = spool.tile([128, 2 * NH], F32, tag="cv")
                    s2 = spool.tile([128, 2 * NH], F32, tag="s2")
                    var = spool.tile([128, 2 * NH], F32, tag="var")
                    m_e = S[:, :, :, 1]
                    m_o = S[:, :, :, 4]
                    cv_e = S[:, :, :, 2]
                    cv_o = S[:, :, :, 5]
                    dm_v = dm[:].rearrange("p (a b) -> p a b", a=2)
                    ms_v = ms[:].rearrange("p (a b) -> p a b", a=2)
                    cv_v = cv[:].rearrange("p (a b) -> p a b", a=2)
                    nc.vector.tensor_sub(dm_v, m_e, m_o)
                    nc.vector.tensor_add(ms_v, m_e, m_o)
                    nc.vector.tensor_add(cv_v, cv_e, cv_o)
                    nc.vector.scalar_tensor_tensor(
                        s2[:], dm[:], 32.0, dm[:], op0=ALU.mult, op1=ALU.mult)
                    nc.vector.tensor_add(var[:], cv[:], s2[:])
                    # rstd = exp(-0.5*ln(var128/127)); nmrs = -0.5*ms*rstd
                    lnv = spool.tile([128, 2 * NH], F32, tag="lnv")
                    rstd = spool.tile([128, 2 * NH], F32, tag="rstd")
                    nmrs = spool.tile([128, 2 * NH], F32, tag="nmrs")
                    nc.scalar.activation(lnv[:], var[:], AF.Ln, scale=1.0 / (D - 1))
                    nc.scalar.activation(rstd[:], lnv[:], AF.Exp, scale=-0.5)
                    nc.vector.scalar_tensor_tensor(
                        nmrs[:], ms[:], -0.5, rstd[:], op0=ALU.mult, op1=ALU.mult)

                    for mi, dstT in ((0, QT), (1, KT)):
                        r = rr[mi]
                        nrm = rpool.tile([128, NH * D], BF16, tag=f"n{mi}")
                        tps = psT.tile([128, NH * D], BF16, tag="tp")
                        for h in range(NH):
                            g = mi * NH + h
                            nc.vector.tensor_scalar(
                                nrm[:, h * D:(h + 1) * D],
                                r[:, h * D:(h + 1) * D],
                                rstd[:, g:g + 1],
                                nmrs[:, g:g + 1],
                                ALU.mult,
                                ALU.add,
                            )
                            nc.tensor.transpose(tps[:, h * D:(h + 1) * D],
                                                nrm[:, h * D:(h + 1) * D], idn[:])
                        # one strided copy moves all 3 transposed heads out
                        dst = dstT[:, :, tt * 128:(tt + 1) * 128]
                        src = tps[:].rearrange("p (h t) -> p h t", h=NH)
                        nc.vector.tensor_copy(dst, src)

            # ---------------- stage 3+4: attention + output projection --------
            with (
                tc.tile_pool(name="att", bufs=3) as apool,
                tc.tile_pool(name="acc", bufs=2) as accpool,
                tc.tile_pool(name="ybuf", bufs=2) as ypool,
                tc.tile_pool(name="obuf", bufs=3) as opool,
                tc.tile_pool(name="psPair", bufs=2, space="PSUM") as psPair,
                tc.tile_pool(name="psY", bufs=2, space="PSUM") as psY,
                tc.tile_pool(name="psD", bufs=1, space="PSUM") as psD,
                tc.tile_pool(name="psP", bufs=1, space="PSUM") as psP,
            ):
                out_r = out.rearrange("(tt p) c -> p tt c", p=128)

                def emit_proj(qc, yTc):
                    # output projection for chunk qc's 4 token tiles
                    for j in range(QC // 128):
                        tt = qc * (QC // 128) + j
                        ot = opool.tile([128, C], BF16, tag="ot")
                        for half in range(2):
                            op_ps = psP.tile([128, C // 2], F32, tag="pp")
                            csl = slice(half * (C // 2), (half + 1) * (C // 2))
                            for h in range(NH):
                                lhs = yTc[:, h, j * 128:(j + 1) * 128]
                                nc.tensor.matmul(op_ps[:], lhs, wp_sb[:, h, csl],
                                                 start=(h == 0), stop=(h == NH - 1))
                            nc.scalar.copy(ot[:, csl], op_ps[:])
                        nc.sync.dma_start(out_r[:, tt, :], ot[:])

                pending = None
                for qc in range(NQC):
                    Q0 = qc * QC
                    n_st = (Q0 + QC) // 128
                    yTc = ypool.tile([128, NH, QC], BF16, tag="yT")  # [d, h, q]
                    for h in range(NH):
                        yps = psY.tile([128, QC], F32, tag="yps")
                        dps = psD.tile([1, QC], F32, tag="dps")
                        for sp in range(n_st // 2):
                            pair = psPair.tile([128, 2 * QC], F32, tag="pair")
                            for j in range(2):
                                st = 2 * sp + j
                                nc.tensor.matmul(
                                    pair[:, j * QC:(j + 1) * QC],
                                    KT[:, h, st * 128:(st + 1) * 128],
                                    QT[:, h, Q0:Q0 + QC],
                                    start=True, stop=True,
                                )
                            et = apool.tile([128, 2 * QC], BF16, tag="et")
                            nc.scalar.activation(et[:], pair[:], AF.Exp, scale=SCALE)
                            for j in range(2):
                                st = 2 * sp + j
                                k = st - n_st + 4
                                if k >= 0:  # diagonal block: zero where s > q
                                    esl = et[:, j * QC:(j + 1) * QC]
                                    nc.vector.tensor_mul(
                                        esl, esl, msk[:, 384 - 128 * k:768 - 128 * k + 128])
                            for j in range(2):
                                st = 2 * sp + j
                                esl = et[:, j * QC:(j + 1) * QC]
                                nc.tensor.matmul(
                                    yps[:],
                                    V[:, st, h * D:(h + 1) * D],
                                    esl,
                                    start=(st == 0), stop=(st == n_st - 1),
                                    skip_group_check=True,
                                )
                                # softmax denominator on PE: ones^T @ exp
                                nc.tensor.matmul(
                                    dps[:1, :],
                                    ones[:],
                                    esl,
                                    start=(st == 0), stop=(st == n_st - 1),
                                    skip_group_check=True,
                                )
                        rc1 = accpool.tile([1, QC], F32, tag="rc1")
                        nc.vector.reciprocal_approx_fast(rc1[:1, :], dps[:1, :])
                        rbc = accpool.tile([128, QC], F32, tag="rbc")
                        nc.gpsimd.partition_broadcast(rbc[:], rc1[:1, :])
                        nc.vector.tensor_mul(yTc[:, h, :], yps[:], rbc[:])
                        if h == 0 and pending is not None:
                            # previous chunk's projection lands here so its
                            # yTc-normalize latency hides under this chunk's
                            # independent attention matmuls
                            emit_proj(*pending)
                            pending = None

                    pending = (qc, yTc)
                emit_proj(*pending)

    nc.compile()
    return nc


def _get_nc():
    if "nc" not in _CACHE:
        _CACHE["nc"] = _build_nc()
    return _CACHE["nc"]


def _bf16(a):
    return np.ascontiguousarray(np.asarray(a, np.float32)).astype(ml_dtypes.bfloat16)


def _in_maps(x, cos, sin, wq, wk, wv, wproj):
    cos = np.asarray(cos, np.float32)
    sin = np.asarray(sin, np.float32)
    cc = np.tile(np.concatenate([cos, cos], axis=1), (1, NH))      # [T, NH*D]
    ss = np.tile(np.concatenate([sin, -sin], axis=1), (1, NH))     # [T, NH*D]
    ident = np.eye(128, dtype=np.float32)
    # msk[p, u] = 1 iff u >= p + 384; diagonal block k uses cols [384-128k, ...)
    p = np.arange(128)[:, None]
    u = np.arange(384 + QC)[None, :]
    mask = (u >= p + 384).astype(np.float32)
    maps = []
    for c in range(8):
        b = c // 2
        hs = (c % 2) * NH
        sl = slice(hs * D, (hs + NH) * D)
        maps.append({
            "xT": _bf16(np.asarray(x[b], np.float32).T),
            "wqT": _bf16(np.asarray(wq, np.float32)[sl].T),
            "wkT": _bf16(np.asarray(wk, np.float32)[sl].T),
            "wvT": _bf16(np.asarray(wv, np.float32)[sl].T),
            "wpT": _bf16(np.asarray(wproj, np.float32).T[sl]),
            "ccin": _bf16(cc),
            "ssin": _bf16(ss),
            "ident": _bf16(ident),
            "ones_in": _bf16(np.ones((128, 1), dtype=np.float32)),
            "mask_in": _bf16(mask),
        })
    return maps


def kernel(x, cos, sin, wq, wk, wv, wproj, _trace=False):
    nc = _get_nc()
    maps = _in_maps(x, cos, sin, wq, wk, wv, wproj)
    res = run_bass_kernel_spmd(nc, maps, core_ids=list(range(8)), trace=_trace)
    parts = [np.asarray(r["out"], dtype=np.float32) for r in res.results]
    outv = np.stack([parts[2 * b] + parts[2 * b + 1] for b in range(B)])
    if _trace:
        _CACHE["last_results"] = res
    return outv.astype(np.float32)
